# revision 37
# baseline (speedup 1.0000x reference)
"""Causal GQA self-attention (B=2, S=2048, D=2048, 32 Q heads / 8 KV heads,
head_dim 64, RoPE) on 8 Trainium2 NeuronCores.

Sharding: data-parallel over batch (2) x tensor-parallel over heads (4).
Core c handles batch c//4 and head group c%4 (8 Q heads, 2 KV heads).
wq/wk/wv column-sharded, wo row-sharded; the 4 partial outputs per batch
are summed on the host at gather time (the "all-reduce").

v3: bf16 matmuls (fp32 HIGH mode streams ~2 cyc/col and drew a 515us HAM
throttle in the fp32 version), software-pipelined proj/attn/outproj
emission, exp batched over head pairs via one 3D-AP ACTIVATE, causal
partial ranges on diagonal tiles, row-packed K=64 score matmuls
(tile_position rows 0/64 run concurrently), V projected k-major directly,
batched-weight DMAs (host supplies partition-major [128, chunk, cols]
layouts so each weight loads in 1-2 DMAs), softmax normalization with
reciprocal_approx_fast at partition 0 (the custom DVE op breaks at
nonzero base partitions) + gpsimd partition_broadcast (no PE in the
normalization path), half1 rows placed via SBUF->SBUF DMA shift.

PSUM budget: scratch(proj+outproj) 2 + score-pairs 2x2 + ppvA 1 + ppvB 1
= 8 banks. Attention paces at the ACT Exp rate (~1.1us per k-tile pair);
projection/outproj matmuls fill the PE gaps under it.
"""

import sys

if "/opt/trn_rl_repo" not in sys.path:
    sys.path.insert(0, "/opt/trn_rl_repo")

import numpy as np
import ml_dtypes

import concourse.bass as bass
import concourse.tile as tile
from concourse import bacc, mybir
from concourse.bass_utils import run_bass_kernel_spmd

B = 2
S = 2048
D = 2048
N_HEAD = 32
N_KV = 8
HD = 64
GROUPS = 4
HQ = N_HEAD // GROUPS
HK = N_KV // GROUPS
QD = HQ * HD
KD = HK * HD
P = 128
SB = 512
NB = S // SB
DC = D // P
QC = QD // P

F32 = mybir.dt.float32
BF16 = mybir.dt.bfloat16
NPBF16 = ml_dtypes.bfloat16

DEBUG_DUMPS = False


def build_kernel():
    nc = bacc.Bacc("TRN2", target_bir_lowering=False, debug=False,
                   num_devices=8)

    xT = nc.dram_tensor("xT", (P, DC, S), BF16, kind="ExternalInput").ap()
    wq = nc.dram_tensor("wq", (P, DC, QD), BF16, kind="ExternalInput").ap()
    wkv = nc.dram_tensor("wkv", (P, DC, KD + KD), BF16,
                         kind="ExternalInput").ap()
    wo = nc.dram_tensor("wo", (P, QC, D), BF16, kind="ExternalInput").ap()
    c4 = nc.dram_tensor("c4", (P, S), BF16, kind="ExternalInput").ap()
    s4 = nc.dram_tensor("s4", (P, S), BF16, kind="ExternalInput").ap()
    cmask = nc.dram_tensor("cmask", (4, P, SB), BF16,
                           kind="ExternalInput").ap()
    outp = nc.dram_tensor("outp", (S, D), BF16, kind="ExternalOutput").ap()
    dumps = None
    if DEBUG_DUMPS:
        dumps = {
            "qtd": nc.dram_tensor("qtd", (QD, S), F32,
                                  kind="ExternalOutput").ap(),
            "ktd": nc.dram_tensor("ktd", (P, S), F32,
                                  kind="ExternalOutput").ap(),
            "vd": nc.dram_tensor("vd", (DC * P, 2 * (HD + 1)), F32,
                                 kind="ExternalOutput").ap(),
            "ad": nc.dram_tensor("ad", (QD, S), F32,
                                 kind="ExternalOutput").ap(),
        }

    with tile.TileContext(nc) as tc, \
         nc.allow_low_precision(reason="bf16 matmul pipeline"):
        _body(nc, tc, xT, wq, wkv, wo, c4, s4, cmask, outp, dumps)

    nc.compile()
    return nc


def _body(nc, tc, xT, wq, wkv, wo, c4, s4, cmask, outp, dumps=None):
    from contextlib import ExitStack

    Exp = mybir.ActivationFunctionType.Exp
    MUL = mybir.AluOpType.mult
    ADD = mybir.AluOpType.add

    ctx = ExitStack()
    with ctx:
        # ---- persistent SBUF pools ----
        qt_pool = ctx.enter_context(tc.tile_pool(name="qt", bufs=QC))
        kt_pool = ctx.enter_context(tc.tile_pool(name="kt", bufs=1))
        vaug_pool = ctx.enter_context(tc.tile_pool(name="vaug", bufs=DC))
        attn_pool = ctx.enter_context(tc.tile_pool(name="attnT", bufs=QC))
        singles = ctx.enter_context(tc.tile_pool(name="singles", bufs=1))
        w_pool = ctx.enter_context(tc.tile_pool(name="weights", bufs=1))
        xt_pool = ctx.enter_context(tc.tile_pool(name="xtp", bufs=3))
        rope_pool = ctx.enter_context(tc.tile_pool(name="ropet", bufs=2))
        exp_pool = ctx.enter_context(tc.tile_pool(name="expp", bufs=3))
        norm_pool = ctx.enter_context(tc.tile_pool(name="normp", bufs=2))
        stage_pool = ctx.enter_context(tc.tile_pool(name="stagep", bufs=3))
        # ---- PSUM pools: 2 + 4 + 1 + 1 = 8 banks ----
        scratch = ctx.enter_context(
            tc.tile_pool(name="scratch", bufs=2, space="PSUM"))
        pat = ctx.enter_context(tc.tile_pool(name="pat", bufs=2,
                                             space="PSUM"))
        ppva_pool = ctx.enter_context(
            tc.tile_pool(name="ppva", bufs=1, space="PSUM"))
        ppvb_pool = ctx.enter_context(
            tc.tile_pool(name="ppvb", bufs=1, space="PSUM"))

        # ---- batched weight loads (x block 0 + wkv/wq first) ----
        xt_tiles = [None] * NB
        xt_sb = [None]

        def load_xt(s, nsplit=4):
            t = xt_pool.tile([P, DC, SB], BF16, tag="xt", name=f"xt{s}")
            step = DC // nsplit
            for q in range(nsplit):
                nc.sync.dma_start(
                    t[:, step * q:step * (q + 1), :],
                    xT[:, step * q:step * (q + 1), s * SB:(s + 1) * SB])
            xt_tiles[s] = t

        # wkv-quarter0 + xt-eighth0 gate the first K matmuls: issue first
        wkv_sb = w_pool.tile([P, DC, KD + KD], BF16, tag="wkv",
                             name="wkv_sb")
        nc.sync.dma_start(wkv_sb[:, 0:4, :], wkv[:, 0:4, :])
        load_xt(0, nsplit=8)
        nc.sync.dma_start(wkv_sb[:, 4:DC, :], wkv[:, 4:DC, :])
        wq_sb = w_pool.tile([P, DC, QD], BF16, tag="wq", name="wq_sb")
        nc.sync.dma_start(wq_sb[:, 0:DC // 2, :], wq[:, 0:DC // 2, :])
        nc.sync.dma_start(wq_sb[:, DC // 2:DC, :], wq[:, DC // 2:DC, :])

        # ---- constants (c4/s4 needed for block-0 rope) ----
        c4_sb = singles.tile([P, S], BF16, tag="c4")
        nc.sync.dma_start(c4_sb[:], c4)
        s4_sb = singles.tile([P, S], BF16, tag="s4")
        nc.sync.dma_start(s4_sb[:], s4)
        load_xt(1)
        cm_sb = []
        for m in range(4):
            t = singles.tile([P, SB], BF16, tag=f"cm{m}", name=f"cm{m}")
            nc.sync.dma_start(t[:], cmask[m])
            cm_sb.append(t)

        # ---- persistent tensors ----
        qt_sb = [qt_pool.tile([P, S], BF16, tag="qt", name=f"qt{c}")
                 for c in range(QC)]
        kt_sb = kt_pool.tile([P, S], BF16, tag="kt")
        vaug_sb = [vaug_pool.tile([P, 2 * (HD + 1)], BF16, tag="vaug",
                                  name=f"vaug{k}") for k in range(DC)]
        for k in range(DC):
            nc.vector.memset(vaug_sb[k][:, HD:HD + 1], 1.0)
            nc.vector.memset(vaug_sb[k][:, 2 * HD + 1:2 * HD + 2], 1.0)
        attn_sb = [attn_pool.tile([P, S], BF16, tag="attnT",
                                  name=f"attnT{c}") for c in range(QC)]

        wo_sb = w_pool.tile([P, QC, D], BF16, tag="wo", name="wo_sb")

        def rope_block(tgt, psrc, sl):
            # evict PSUM -> bf16 slice of tgt, then rotate in place
            nc.scalar.copy(tgt[:, sl], psrc[:])
            sw = rope_pool.tile([P, SB], BF16, tag="sw")
            m1 = rope_pool.tile([P, SB], BF16, tag="m1")
            hw = HD // 2
            for b in range(0, P, hw):
                sb2 = b + hw if (b // hw) % 2 == 0 else b - hw
                nc.sync.dma_start(sw[b:b + hw, :], tgt[sb2:sb2 + hw, sl])
            # all rope math on DVE: gpsimd must run ONLY partition_broadcast
            # (mixing ops forces a ~6us ucode library swap per alternation)
            nc.vector.tensor_tensor(m1[:], tgt[:, sl], c4_sb[:, sl], MUL)
            nc.vector.tensor_tensor(sw[:], sw[:], s4_sb[:, sl], MUL)
            nc.vector.tensor_tensor(tgt[:, sl], m1[:], sw[:], ADD)

        def proj_k(s):
            sl = slice(s * SB, (s + 1) * SB)
            pk = scratch.tile([P, SB], F32, tag="scr", name=f"pk{s}")
            xt = xt_tiles[s]
            for d in range(DC):
                nc.tensor.matmul(pk[:], wkv_sb[:, d, 0:KD], xt[:, d, :],
                                 start=(d == 0), stop=(d == DC - 1))
            rope_block(kt_sb, pk, sl)

        def proj_q(s, c):
            sl = slice(s * SB, (s + 1) * SB)
            pq = scratch.tile([P, SB], F32, tag="scr", name=f"pq{s}_{c}")
            xt = xt_tiles[s]
            for d in range(DC):
                nc.tensor.matmul(pq[:], wq_sb[:, d, c * P:(c + 1) * P],
                                 xt[:, d, :], start=(d == 0),
                                 stop=(d == DC - 1))
            rope_block(qt_sb[c], pq, sl)

        def proj_q_quad(s, c, qd, state):
            # 4 accumulation matmuls; qd==0 allocates, qd==3 evicts+ropes
            if qd == 0:
                state[c] = scratch.tile([P, SB], F32, tag="scr",
                                        name=f"pq{s}_{c}")
            pq = state[c]
            xt = xt_tiles[s]
            for d in range(4 * qd, 4 * qd + 4):
                nc.tensor.matmul(pq[:], wq_sb[:, d, c * P:(c + 1) * P],
                                 xt[:, d, :], start=(d == 0),
                                 stop=(d == DC - 1))
            if qd == 3:
                rope_block(qt_sb[c], pq, slice(s * SB, (s + 1) * SB))

        def proj_k_quad(s, qd, state):
            if qd == 0:
                state["k"] = scratch.tile([P, SB], F32, tag="scr",
                                          name=f"pk{s}")
            pk = state["k"]
            xt = xt_tiles[s]
            for d in range(4 * qd, 4 * qd + 4):
                nc.tensor.matmul(pk[:], wkv_sb[:, d, 0:KD], xt[:, d, :],
                                 start=(d == 0), stop=(d == DC - 1))
            if qd == 3:
                rope_block(kt_sb, pk, slice(s * SB, (s + 1) * SB))

        def proj_v_half(s, t4, h, state):
            # V[k, dv] k-major: lhsT = xT s-subchunk, rhs = wv; two halves
            if h == 0:
                state[t4] = scratch.tile([P, P], F32, tag="scr",
                                         name=f"pv{s}_{t4}")
            pv = state[t4]
            xt = xt_tiles[s]
            for d in range(8 * h, 8 * h + 8):
                nc.tensor.matmul(
                    pv[:], xt[:, d, t4 * P:(t4 + 1) * P],
                    wkv_sb[:, d, KD:2 * KD],
                    start=(d == 0), stop=(d == DC - 1))
            if h == 1:
                vt = vaug_sb[4 * s + t4]
                # early windows are DVE-bound, late windows ACT-bound
                if s <= 1:
                    nc.scalar.copy(vt[:, 0:HD], pv[:, 0:HD])
                    nc.scalar.copy(vt[:, HD + 1:2 * HD + 1],
                                   pv[:, HD:2 * HD])
                else:
                    nc.vector.tensor_copy(vt[:, 0:HD], pv[:, 0:HD])
                    nc.vector.tensor_copy(vt[:, HD + 1:2 * HD + 1],
                                          pv[:, HD:2 * HD])

        def attn_chunk(qb, c, last=False, fill=None):
            # heads A=(c, half0) rows 0:64, B=(c, half1) rows 64:128
            nk = 4 * qb + 4
            qsl = slice(qb * SB, (qb + 1) * SB)
            ppvA = ppva_pool.tile([P, SB], F32, tag="ppva",
                                  name=f"ppvA{qb}_{c}")
            ppvB = ppvb_pool.tile([P, SB], F32, tag="ppvb",
                                  name=f"ppvB{qb}_{c}")
            for j in range(nk):
                if fill is not None:
                    fill(j)
                m = j - 4 * qb
                qs = max(m, 0) * P  # valid q start within the block
                ps = pat.tile([P, 2, SB], F32, tag="pat",
                              name=f"ps{qb}_{c}_{j}")
                ea = exp_pool.tile([P, 2, SB], BF16, tag="ea",
                                   name=f"ea{qb}_{c}_{j}")
                kcols = slice(j * P, (j + 1) * P)
                qcols = slice(qb * SB + qs, (qb + 1) * SB)
                # row-packed score MMs: A on array rows 0:64, B on 64:128
                nc.tensor.matmul(ps[:, 0, qs:], kt_sb[0:HD, kcols],
                                 qt_sb[c][0:HD, qcols],
                                 start=True, stop=True)
                nc.tensor.matmul(ps[:, 1, qs:], kt_sb[HD:P, kcols],
                                 qt_sb[c][HD:P, qcols],
                                 start=True, stop=True)
                # one Exp over both heads' valid range
                nc.scalar.activation(ea[:, :, qs:], ps[:, :, qs:], Exp)
                if m >= 0:
                    nc.vector.tensor_tensor(ea[:, 0, qs:], ea[:, 0, qs:],
                                            cm_sb[m][:, qs:], MUL)
                    nc.vector.tensor_tensor(ea[:, 1, qs:], ea[:, 1, qs:],
                                            cm_sb[m][:, qs:], MUL)
                nc.tensor.matmul(ppvA[0:HD + 1, qs:],
                                 vaug_sb[j][:, 0:HD + 1], ea[:, 0, qs:],
                                 start=(j == 0), stop=(j == nk - 1))
                nc.tensor.matmul(ppvB[0:HD + 1, qs:],
                                 vaug_sb[j][:, HD + 1:2 * (HD + 1)],
                                 ea[:, 1, qs:],
                                 start=(j == 0), stop=(j == nk - 1))
            # ---- normalize (no PE involvement) ----
            # evict ppv FIRST (av + ssum) so the single-buffered ppv banks
            # release early; the recip/broadcast chain then runs off SBUF
            evs = []
            for half, ppv in ((0, ppvA), (1, ppvB)):
                # one f32 eviction covers values (0:64) and sums (row 64)
                av = norm_pool.tile([P, SB], F32, tag="av",
                                    name=f"av{half}")
                if last:
                    # tail: ACT is idle after the final exp; shorten the
                    # DVE critical path into outproj(3)
                    nc.scalar.copy(av[0:HD + 1, :], ppv[0:HD + 1, :])
                else:
                    nc.vector.tensor_copy(av[0:HD + 1, :], ppv[0:HD + 1, :])
                evs.append((half, av))
            for half, av in evs:
                # sums sit at partition 64; reciprocal_approx_fast only
                # works at base partition 0 -> relocate via SBUF DMA
                ssum0 = norm_pool.tile([P, SB], F32, tag="ssum0")
                nc.sync.dma_start(ssum0[0:1, :], av[HD:HD + 1, :])
                rc = norm_pool.tile([P, SB], F32, tag="rc")
                nc.vector.reciprocal_approx_fast(rc[0:1, :], ssum0[0:1, :])
                bcs = norm_pool.tile([P, SB], F32, tag="bcs")
                nc.gpsimd.partition_broadcast(bcs[0:HD, :], rc[0:1, :])
                if half == 0:
                    nc.vector.tensor_tensor(attn_sb[c][0:HD, qsl],
                                            bcs[0:HD, :], av[0:HD, :], MUL)
                else:
                    stn = norm_pool.tile([P, SB], BF16, tag="stn")
                    nc.vector.tensor_tensor(stn[0:HD, :], bcs[0:HD, :],
                                            av[0:HD, :], MUL)
                    # partition shift 0:64 -> 64:128 via SBUF-to-SBUF DMA
                    nc.sync.dma_start(attn_sb[c][HD:P, qsl], stn[0:HD, :])

        def outproj_unit(sc, ob, tail=False):
            po = scratch.tile([P, SB], F32, tag="scr", name=f"po{sc}_{ob}")
            for c in range(QC):
                nc.tensor.matmul(po[:],
                                 attn_sb[c][:, sc * P:(sc + 1) * P],
                                 wo_sb[:, c, ob * SB:(ob + 1) * SB],
                                 start=(c == 0), stop=(c == QC - 1))
            stg = stage_pool.tile([P, SB], BF16, tag="stg")
            # qb0 runs in the DVE-bound s1 window -> ACT; later qbs run in
            # ACT-bound windows -> DVE; tail alternates (both have slack)
            if (tail and (sc + ob) % 2 == 0) or (not tail and sc < 4):
                nc.scalar.copy(stg[:], po[:])
            else:
                nc.vector.tensor_copy(stg[:], po[:])
            nc.sync.dma_start(
                outp[sc * P:(sc + 1) * P, ob * SB:(ob + 1) * SB], stg[:])

        # ================= pipelined emission =================
        # Global fill-task queues drained one task per attention k-tile.
        # `crit` tasks (projections, incl. the NEXT window's K/Q0) run in
        # strict list order so the 2-buf scratch pool rotation stays
        # hazard-free; `defer` tasks (outproj units) only run once crit is
        # empty, letting PE work spill from PE-bound early windows into
        # the ACT-bound late windows.
        crit, defer = [], []

        def pop_one():
            if crit:
                crit.pop(0)[1]()
            elif defer:
                defer.pop(0)[1]()

        def flush(label):
            idxs = [i for i, (lb, _) in enumerate(crit) if lb == label]
            if not idxs:
                return
            for _ in range(idxs[-1] + 1):
                crit.pop(0)[1]()

        proj_k(0)
        proj_q(0, 0)
        for s in range(NB):
            qstate, vstate = {}, {}
            nstate = {}

            def vtasks(s=s, vstate=vstate):
                for t4 in range(4):
                    for h in range(2):
                        crit.append((f"v_{s}", lambda t4=t4, h=h:
                                     proj_v_half(s, t4, h, vstate)))

            def qtasks(c, s=s, qstate=qstate):
                for qd in range(4):
                    crit.append((f"q{c}_{s}", lambda c=c, qd=qd:
                                 proj_q_quad(s, c, qd, qstate)))

            if s == 0:
                vtasks()
                qtasks(1)
            else:
                qtasks(1)
                vtasks()
            qtasks(2)
            qtasks(3)
            if s + 1 < NB:
                for qd in range(4):
                    crit.append((f"k_{s + 1}", lambda qd=qd, s1=s + 1,
                                 nst=nstate: proj_k_quad(s1, qd, nst)))
                for qd in range(4):
                    crit.append((f"q0_{s + 1}", lambda qd=qd, s1=s + 1,
                                 nst=nstate: proj_q_quad(s1, 0, qd, nst)))
            if s >= 1:
                for u in range(16):
                    defer.append(("o", lambda u=u, s=s:
                                  outproj_unit(4 * (s - 1) + u // 4,
                                               u % 4)))

            def fill_one(j, s=s):
                if j == 4 * s:
                    # diagonal k-tiles need this window's V: flush it
                    flush(f"v_{s}")
                pop_one()

            flush(f"q0_{s}")  # leftovers of this window's K/Q0 tasks
            attn_chunk(s, 0, fill=fill_one)
            flush(f"q1_{s}")
            attn_chunk(s, 1, fill=fill_one)
            flush(f"q2_{s}")
            attn_chunk(s, 2, fill=fill_one)
            flush(f"q3_{s}")
            attn_chunk(s, 3, last=(s == NB - 1), fill=fill_one)
            if s == 0:
                # wo needed from outproj(0) in the s=1 window
                nc.sync.dma_start(wo_sb[:, 0:2, :], wo[:, 0:2, :])
                nc.sync.dma_start(wo_sb[:, 2:4, :], wo[:, 2:4, :])
            if s + 2 < NB:
                load_xt(s + 2)
        while crit or defer:
            pop_one()
        for c in range(QC):
            for ob in range(4):
                outproj_unit(4 * (NB - 1) + c, ob, tail=True)

        if dumps is not None:
            for c in range(QC):
                dq = stage_pool.tile([P, S], F32, tag="dump", bufs=1,
                                     name=f"dq{c}")
                nc.vector.tensor_copy(dq[:], qt_sb[c][:])
                nc.sync.dma_start(dumps["qtd"][c * P:(c + 1) * P, :], dq[:])
                da = stage_pool.tile([P, S], F32, tag="dump", bufs=1,
                                     name=f"da{c}")
                nc.vector.tensor_copy(da[:], attn_sb[c][:])
                nc.sync.dma_start(dumps["ad"][c * P:(c + 1) * P, :], da[:])
            dk = stage_pool.tile([P, S], F32, tag="dump", bufs=1,
                                 name="dk")
            nc.vector.tensor_copy(dk[:], kt_sb[:])
            nc.sync.dma_start(dumps["ktd"][:], dk[:])
            for k in range(DC):
                dv = stage_pool.tile([P, 2 * (HD + 1)], F32, tag="dump",
                                     bufs=1, name=f"dv{k}")
                nc.vector.tensor_copy(dv[:], vaug_sb[k][:])
                nc.sync.dma_start(dumps["vd"][k * P:(k + 1) * P, :], dv[:])


_NC_CACHE = None


def _get_nc():
    global _NC_CACHE
    if _NC_CACHE is None:
        _NC_CACHE = build_kernel()
    return _NC_CACHE


def _deinterleave_cols(w):
    """Per 64-col head block: reorder cols to [evens(real), odds(imag)]."""
    d, n = w.shape
    out = np.empty_like(w)
    for h0 in range(0, n, HD):
        blk = w[:, h0:h0 + HD]
        out[:, h0:h0 + HD // 2] = blk[:, 0::2]
        out[:, h0 + HD // 2:h0 + HD] = blk[:, 1::2]
    return out


def _chunk_major(w, ncols):
    """(D_rows, ncols) -> (128, D_rows//128, ncols) partition-major bf16."""
    d = w.shape[0]
    return np.ascontiguousarray(
        w.reshape(d // P, P, ncols).transpose(1, 0, 2)).astype(NPBF16)


def _prep_inputs(x, wq, wk, wv, wo, freqs_cos, freqs_sin):
    scale = 1.0 / np.sqrt(HD)
    cosT = np.ascontiguousarray(freqs_cos[:S].T.astype(np.float32))  # (32,S)
    sinT = np.ascontiguousarray(freqs_sin[:S].T.astype(np.float32))
    c4 = np.tile(cosT, (4, 1)).astype(NPBF16)                  # (128, S)
    s4 = np.concatenate([-sinT, sinT, -sinT, sinT], 0).astype(NPBF16)
    kk = np.arange(P, dtype=np.int64)[:, None]
    qq = np.arange(SB, dtype=np.int64)[None, :]
    cmask = np.stack(
        [(kk <= qq - P * m).astype(np.float32) for m in range(4)]
    ).astype(NPBF16)

    xTs = [_chunk_major(np.ascontiguousarray(x[b].T), S) for b in range(B)]
    per_group = []
    for g in range(GROUPS):
        wq_full = np.ascontiguousarray(wq[:, g * QD:(g + 1) * QD])
        # chunk c holds heads [c, c+4] so q-head halves align with kv halves
        order = []
        for c in range(QC):
            order.extend(range(c * HD, (c + 1) * HD))
            order.extend(range((c + 4) * HD, (c + 5) * HD))
        wq_g = _deinterleave_cols(wq_full[:, order]) * scale
        wk_g = _deinterleave_cols(
            np.ascontiguousarray(wk[:, g * KD:(g + 1) * KD]))
        wv_g = np.ascontiguousarray(wv[:, g * KD:(g + 1) * KD])
        wkv_g = np.concatenate([wk_g, wv_g], axis=1)
        wo_g = np.ascontiguousarray(wo[g * QD:(g + 1) * QD, :][order, :])
        per_group.append((_chunk_major(wq_g, QD),
                          _chunk_major(wkv_g, KD + KD),
                          _chunk_major(wo_g, D)))

    in_maps = []
    for core in range(8):
        b, g = core // GROUPS, core % GROUPS
        wq_g, wkv_g, wo_g = per_group[g]
        in_maps.append({
            "xT": xTs[b],
            "wq": wq_g,
            "wkv": wkv_g,
            "wo": wo_g,
            "c4": c4,
            "s4": s4,
            "cmask": cmask,
        })
    return in_maps


def kernel(x, wq, wk, wv, wo, freqs_cos, freqs_sin, _trace=False):
    nc = _get_nc()
    in_maps = _prep_inputs(np.asarray(x, dtype=np.float32),
                           np.asarray(wq, dtype=np.float32),
                           np.asarray(wk, dtype=np.float32),
                           np.asarray(wv, dtype=np.float32),
                           np.asarray(wo, dtype=np.float32),
                           np.asarray(freqs_cos, dtype=np.float32),
                           np.asarray(freqs_sin, dtype=np.float32))
    res = run_bass_kernel_spmd(nc, in_maps, core_ids=list(range(8)),
                               trace=_trace)
    out = np.zeros((B, S, D), dtype=np.float32)
    for core in range(8):
        out[core // GROUPS] += res.results[core]["outp"].astype(np.float32)
    if _trace:
        kernel.last_results = res
    return out


# revision 40
# speedup vs baseline: 1.1935x; 1.1935x over previous
"""Causal GQA self-attention (B=2, S=2048, D=2048, 32 Q heads / 8 KV heads,
head_dim 64, RoPE) on 8 Trainium2 NeuronCores.

Sharding: data-parallel over batch (2) x tensor-parallel over heads (4).
Core c handles batch c//4 and head group c%4 (8 Q heads, 2 KV heads).
wq/wk/wv column-sharded, wo row-sharded; the 4 partial outputs per batch
are summed on the host at gather time (the "all-reduce").

v3: bf16 matmuls (fp32 HIGH mode streams ~2 cyc/col and drew a 515us HAM
throttle in the fp32 version), software-pipelined proj/attn/outproj
emission, exp batched over head pairs via one 3D-AP ACTIVATE, causal
partial ranges on diagonal tiles, row-packed K=64 score matmuls
(tile_position rows 0/64 run concurrently), V projected k-major directly,
batched-weight DMAs (host supplies partition-major [128, chunk, cols]
layouts so each weight loads in 1-2 DMAs), softmax normalization with
reciprocal_approx_fast at partition 0 (the custom DVE op breaks at
nonzero base partitions) + gpsimd partition_broadcast (no PE in the
normalization path), half1 rows placed via SBUF->SBUF DMA shift.

PSUM budget: scratch(proj+outproj) 2 + score-pairs 2x2 + ppvA 1 + ppvB 1
= 8 banks. Attention paces at the ACT Exp rate (~1.1us per k-tile pair);
projection/outproj matmuls fill the PE gaps under it.
"""

import sys

if "/opt/trn_rl_repo" not in sys.path:
    sys.path.insert(0, "/opt/trn_rl_repo")

import numpy as np
import ml_dtypes

import concourse.bass as bass
import concourse.tile as tile
from concourse import bacc, mybir
from concourse.bass_utils import run_bass_kernel_spmd

B = 2
S = 2048
D = 2048
N_HEAD = 32
N_KV = 8
HD = 64
GROUPS = 4
HQ = N_HEAD // GROUPS
HK = N_KV // GROUPS
QD = HQ * HD
KD = HK * HD
P = 128
SB = 512
NB = S // SB
DC = D // P
QC = QD // P

F32 = mybir.dt.float32
BF16 = mybir.dt.bfloat16
NPBF16 = ml_dtypes.bfloat16

DEBUG_DUMPS = False


def build_kernel():
    nc = bacc.Bacc("TRN2", target_bir_lowering=False, debug=False,
                   num_devices=8)

    xT = nc.dram_tensor("xT", (P, DC, S), BF16, kind="ExternalInput").ap()
    wq = nc.dram_tensor("wq", (P, DC, QD), BF16, kind="ExternalInput").ap()
    wkv = nc.dram_tensor("wkv", (P, DC, KD + KD), BF16,
                         kind="ExternalInput").ap()
    wo = nc.dram_tensor("wo", (P, QC, D), BF16, kind="ExternalInput").ap()
    c4 = nc.dram_tensor("c4", (P, S), BF16, kind="ExternalInput").ap()
    s4 = nc.dram_tensor("s4", (P, S), BF16, kind="ExternalInput").ap()
    cmask = nc.dram_tensor("cmask", (4, P, SB), BF16,
                           kind="ExternalInput").ap()
    outp = nc.dram_tensor("outp", (S, D), BF16, kind="ExternalOutput").ap()
    dumps = None
    if DEBUG_DUMPS:
        dumps = {
            "qtd": nc.dram_tensor("qtd", (QD, S), F32,
                                  kind="ExternalOutput").ap(),
            "ktd": nc.dram_tensor("ktd", (P, S), F32,
                                  kind="ExternalOutput").ap(),
            "vd": nc.dram_tensor("vd", (DC * P, 2 * (HD + 1)), F32,
                                 kind="ExternalOutput").ap(),
            "ad": nc.dram_tensor("ad", (QD, S), F32,
                                 kind="ExternalOutput").ap(),
        }

    with tile.TileContext(nc) as tc, \
         nc.allow_low_precision(reason="bf16 matmul pipeline"):
        _body(nc, tc, xT, wq, wkv, wo, c4, s4, cmask, outp, dumps)

    nc.compile()
    return nc


def _body(nc, tc, xT, wq, wkv, wo, c4, s4, cmask, outp, dumps=None):
    from contextlib import ExitStack

    Exp = mybir.ActivationFunctionType.Exp
    MUL = mybir.AluOpType.mult
    ADD = mybir.AluOpType.add

    ctx = ExitStack()
    with ctx:
        # ---- persistent SBUF pools ----
        qt_pool = ctx.enter_context(tc.tile_pool(name="qt", bufs=QC))
        kt_pool = ctx.enter_context(tc.tile_pool(name="kt", bufs=1))
        vaug_pool = ctx.enter_context(tc.tile_pool(name="vaug", bufs=DC))
        attn_pool = ctx.enter_context(tc.tile_pool(name="attnT", bufs=QC))
        singles = ctx.enter_context(tc.tile_pool(name="singles", bufs=1))
        w_pool = ctx.enter_context(tc.tile_pool(name="weights", bufs=1))
        xt_pool = ctx.enter_context(tc.tile_pool(name="xtp", bufs=3))
        rope_pool = ctx.enter_context(tc.tile_pool(name="ropet", bufs=2))
        exp_pool = ctx.enter_context(tc.tile_pool(name="expp", bufs=3))
        norm_pool = ctx.enter_context(tc.tile_pool(name="normp", bufs=2))
        stage_pool = ctx.enter_context(tc.tile_pool(name="stagep", bufs=3))
        # ---- PSUM pools: 2 + 4 + 1 + 1 = 8 banks ----
        scratch = ctx.enter_context(
            tc.tile_pool(name="scratch", bufs=2, space="PSUM"))
        pat = ctx.enter_context(tc.tile_pool(name="pat", bufs=2,
                                             space="PSUM"))
        ppva_pool = ctx.enter_context(
            tc.tile_pool(name="ppva", bufs=1, space="PSUM"))
        ppvb_pool = ctx.enter_context(
            tc.tile_pool(name="ppvb", bufs=1, space="PSUM"))

        # ---- batched weight loads (x block 0 + wkv/wq first) ----
        xt_tiles = [None] * NB
        xt_sb = [None]

        def load_xt(s, nsplit=4):
            t = xt_pool.tile([P, DC, SB], BF16, tag="xt", name=f"xt{s}")
            step = DC // nsplit
            for q in range(nsplit):
                nc.sync.dma_start(
                    t[:, step * q:step * (q + 1), :],
                    xT[:, step * q:step * (q + 1), s * SB:(s + 1) * SB])
            xt_tiles[s] = t

        # wkv-quarter0 + xt-eighth0 gate the first K matmuls: issue first
        wkv_sb = w_pool.tile([P, DC, KD + KD], BF16, tag="wkv",
                             name="wkv_sb")
        nc.sync.dma_start(wkv_sb[:, 0:4, :], wkv[:, 0:4, :])
        load_xt(0, nsplit=8)
        nc.sync.dma_start(wkv_sb[:, 4:DC, :], wkv[:, 4:DC, :])
        wq_sb = w_pool.tile([P, DC, QD], BF16, tag="wq", name="wq_sb")
        nc.sync.dma_start(wq_sb[:, 0:DC // 2, :], wq[:, 0:DC // 2, :])
        nc.sync.dma_start(wq_sb[:, DC // 2:DC, :], wq[:, DC // 2:DC, :])

        # ---- constants (c4/s4 needed for block-0 rope) ----
        c4_sb = singles.tile([P, S], BF16, tag="c4")
        nc.sync.dma_start(c4_sb[:], c4)
        s4_sb = singles.tile([P, S], BF16, tag="s4")
        nc.sync.dma_start(s4_sb[:], s4)
        load_xt(1)
        cm_sb = []
        for m in range(4):
            t = singles.tile([P, SB], BF16, tag=f"cm{m}", name=f"cm{m}")
            nc.sync.dma_start(t[:], cmask[m])
            cm_sb.append(t)

        # ---- persistent tensors ----
        qt_sb = [qt_pool.tile([P, S], BF16, tag="qt", name=f"qt{c}")
                 for c in range(QC)]
        kt_sb = kt_pool.tile([P, S], BF16, tag="kt")
        vaug_sb = [vaug_pool.tile([P, 2 * (HD + 1)], BF16, tag="vaug",
                                  name=f"vaug{k}") for k in range(DC)]
        for k in range(DC):
            nc.vector.memset(vaug_sb[k][:, HD:HD + 1], 1.0)
            nc.vector.memset(vaug_sb[k][:, 2 * HD + 1:2 * HD + 2], 1.0)
        attn_sb = [attn_pool.tile([P, S], BF16, tag="attnT",
                                  name=f"attnT{c}") for c in range(QC)]

        wo_sb = w_pool.tile([P, QC, D], BF16, tag="wo", name="wo_sb")

        def rope_block(tgt, psrc, sl):
            # evict PSUM -> bf16 slice of tgt, then rotate in place
            nc.scalar.copy(tgt[:, sl], psrc[:])
            sw = rope_pool.tile([P, SB], BF16, tag="sw")
            m1 = rope_pool.tile([P, SB], BF16, tag="m1")
            hw = HD // 2
            for b in range(0, P, hw):
                sb2 = b + hw if (b // hw) % 2 == 0 else b - hw
                nc.sync.dma_start(sw[b:b + hw, :], tgt[sb2:sb2 + hw, sl])
            # all rope math on DVE: gpsimd must run ONLY partition_broadcast
            # (mixing ops forces a ~6us ucode library swap per alternation)
            nc.vector.tensor_tensor(m1[:], tgt[:, sl], c4_sb[:, sl], MUL)
            nc.vector.tensor_tensor(sw[:], sw[:], s4_sb[:, sl], MUL)
            nc.vector.tensor_tensor(tgt[:, sl], m1[:], sw[:], ADD)

        def proj_k(s):
            sl = slice(s * SB, (s + 1) * SB)
            pk = scratch.tile([P, SB], F32, tag="scr", name=f"pk{s}")
            xt = xt_tiles[s]
            for d in range(DC):
                nc.tensor.matmul(pk[:], wkv_sb[:, d, 0:KD], xt[:, d, :],
                                 start=(d == 0), stop=(d == DC - 1))
            rope_block(kt_sb, pk, sl)

        def proj_q(s, c):
            sl = slice(s * SB, (s + 1) * SB)
            pq = scratch.tile([P, SB], F32, tag="scr", name=f"pq{s}_{c}")
            xt = xt_tiles[s]
            for d in range(DC):
                nc.tensor.matmul(pq[:], wq_sb[:, d, c * P:(c + 1) * P],
                                 xt[:, d, :], start=(d == 0),
                                 stop=(d == DC - 1))
            rope_block(qt_sb[c], pq, sl)

        def proj_q_quad(s, c, qd, state):
            # 4 accumulation matmuls; qd==0 allocates, qd==3 evicts+ropes
            if qd == 0:
                state[c] = scratch.tile([P, SB], F32, tag="scr",
                                        name=f"pq{s}_{c}")
            pq = state[c]
            xt = xt_tiles[s]
            for d in range(4 * qd, 4 * qd + 4):
                nc.tensor.matmul(pq[:], wq_sb[:, d, c * P:(c + 1) * P],
                                 xt[:, d, :], start=(d == 0),
                                 stop=(d == DC - 1))
            if qd == 3:
                rope_block(qt_sb[c], pq, slice(s * SB, (s + 1) * SB))

        def proj_k_quad(s, qd, state):
            if qd == 0:
                state["k"] = scratch.tile([P, SB], F32, tag="scr",
                                          name=f"pk{s}")
            pk = state["k"]
            xt = xt_tiles[s]
            for d in range(4 * qd, 4 * qd + 4):
                nc.tensor.matmul(pk[:], wkv_sb[:, d, 0:KD], xt[:, d, :],
                                 start=(d == 0), stop=(d == DC - 1))
            if qd == 3:
                rope_block(kt_sb, pk, slice(s * SB, (s + 1) * SB))

        def proj_v_half(s, t4, h, state):
            # V[k, dv] k-major: lhsT = xT s-subchunk, rhs = wv; two halves
            if h == 0:
                state[t4] = scratch.tile([P, P], F32, tag="scr",
                                         name=f"pv{s}_{t4}")
            pv = state[t4]
            xt = xt_tiles[s]
            for d in range(8 * h, 8 * h + 8):
                nc.tensor.matmul(
                    pv[:], xt[:, d, t4 * P:(t4 + 1) * P],
                    wkv_sb[:, d, KD:2 * KD],
                    start=(d == 0), stop=(d == DC - 1))
            if h == 1:
                vt = vaug_sb[4 * s + t4]
                # early windows are DVE-bound, late windows ACT-bound
                if s <= 1:
                    nc.scalar.copy(vt[:, 0:HD], pv[:, 0:HD])
                    nc.scalar.copy(vt[:, HD + 1:2 * HD + 1],
                                   pv[:, HD:2 * HD])
                else:
                    nc.vector.tensor_copy(vt[:, 0:HD], pv[:, 0:HD])
                    nc.vector.tensor_copy(vt[:, HD + 1:2 * HD + 1],
                                          pv[:, HD:2 * HD])

        def attn_chunk(qb, c, last=False, fill=None):
            # heads A=(c, half0) rows 0:64, B=(c, half1) rows 64:128
            nk = 4 * qb + 4
            qsl = slice(qb * SB, (qb + 1) * SB)
            ppvA = ppva_pool.tile([P, SB], F32, tag="ppva",
                                  name=f"ppvA{qb}_{c}")
            ppvB = ppvb_pool.tile([P, SB], F32, tag="ppvb",
                                  name=f"ppvB{qb}_{c}")
            for j in range(nk):
                if fill is not None:
                    fill(j)
                m = j - 4 * qb
                qs = max(m, 0) * P  # valid q start within the block
                ps = pat.tile([P, 2, SB], F32, tag="pat",
                              name=f"ps{qb}_{c}_{j}")
                ea = exp_pool.tile([P, 2, SB], BF16, tag="ea",
                                   name=f"ea{qb}_{c}_{j}")
                kcols = slice(j * P, (j + 1) * P)
                qcols = slice(qb * SB + qs, (qb + 1) * SB)
                # row-packed score MMs: A on array rows 0:64, B on 64:128
                nc.tensor.matmul(ps[:, 0, qs:], kt_sb[0:HD, kcols],
                                 qt_sb[c][0:HD, qcols],
                                 start=True, stop=True)
                nc.tensor.matmul(ps[:, 1, qs:], kt_sb[HD:P, kcols],
                                 qt_sb[c][HD:P, qcols],
                                 start=True, stop=True)
                # one Exp over both heads' valid range
                nc.scalar.activation(ea[:, :, qs:], ps[:, :, qs:], Exp)
                if m >= 0:
                    nc.vector.tensor_tensor(ea[:, 0, qs:], ea[:, 0, qs:],
                                            cm_sb[m][:, qs:], MUL)
                    nc.vector.tensor_tensor(ea[:, 1, qs:], ea[:, 1, qs:],
                                            cm_sb[m][:, qs:], MUL)
                nc.tensor.matmul(ppvA[0:HD + 1, qs:],
                                 vaug_sb[j][:, 0:HD + 1], ea[:, 0, qs:],
                                 start=(j == 0), stop=(j == nk - 1))
                nc.tensor.matmul(ppvB[0:HD + 1, qs:],
                                 vaug_sb[j][:, HD + 1:2 * (HD + 1)],
                                 ea[:, 1, qs:],
                                 start=(j == 0), stop=(j == nk - 1))
            # ---- normalize (no PE involvement) ----
            # evict ppv FIRST (av + ssum) so the single-buffered ppv banks
            # release early; the recip/broadcast chain then runs off SBUF
            evs = []
            for half, ppv in ((0, ppvA), (1, ppvB)):
                # one f32 eviction covers values (0:64) and sums (row 64)
                av = norm_pool.tile([P, SB], F32, tag="av",
                                    name=f"av{half}")
                if last:
                    # tail: ACT is idle after the final exp; shorten the
                    # DVE critical path into outproj(3)
                    nc.scalar.copy(av[0:HD + 1, :], ppv[0:HD + 1, :])
                else:
                    nc.vector.tensor_copy(av[0:HD + 1, :], ppv[0:HD + 1, :])
                evs.append((half, av))
            for half, av in evs:
                # sums sit at partition 64; reciprocal_approx_fast only
                # works at base partition 0 -> relocate via SBUF DMA
                ssum0 = norm_pool.tile([P, SB], F32, tag="ssum0")
                nc.sync.dma_start(ssum0[0:1, :], av[HD:HD + 1, :])
                rc = norm_pool.tile([P, SB], F32, tag="rc")
                nc.vector.reciprocal_approx_fast(rc[0:1, :], ssum0[0:1, :])
                bcs = norm_pool.tile([P, SB], F32, tag="bcs")
                nc.gpsimd.partition_broadcast(bcs[0:HD, :], rc[0:1, :])
                if half == 0:
                    nc.vector.tensor_tensor(attn_sb[c][0:HD, qsl],
                                            bcs[0:HD, :], av[0:HD, :], MUL)
                else:
                    stn = norm_pool.tile([P, SB], BF16, tag="stn")
                    nc.vector.tensor_tensor(stn[0:HD, :], bcs[0:HD, :],
                                            av[0:HD, :], MUL)
                    # partition shift 0:64 -> 64:128 via SBUF-to-SBUF DMA
                    nc.sync.dma_start(attn_sb[c][HD:P, qsl], stn[0:HD, :])

        def outproj_unit(sc, ob, tail=False):
            po = scratch.tile([P, SB], F32, tag="scr", name=f"po{sc}_{ob}")
            for c in range(QC):
                nc.tensor.matmul(po[:],
                                 attn_sb[c][:, sc * P:(sc + 1) * P],
                                 wo_sb[:, c, ob * SB:(ob + 1) * SB],
                                 start=(c == 0), stop=(c == QC - 1))
            stg = stage_pool.tile([P, SB], BF16, tag="stg")
            # mid-kernel units run inside the ACT-paced exp stream: never
            # put their evictions on ACT; tail alternates (both idle)
            if tail and (sc + ob) % 2 == 0:
                nc.scalar.copy(stg[:], po[:])
            else:
                nc.vector.tensor_copy(stg[:], po[:])
            nc.sync.dma_start(
                outp[sc * P:(sc + 1) * P, ob * SB:(ob + 1) * SB], stg[:])

        # ================= pipelined emission =================
        # Global fill-task queues drained one task per attention k-tile.
        # `crit` tasks (projections, incl. the NEXT window's K/Q0) run in
        # strict list order so the 2-buf scratch pool rotation stays
        # hazard-free; `defer` tasks (outproj units) only run once crit is
        # empty, letting PE work spill from PE-bound early windows into
        # the ACT-bound late windows.
        crit, defer = [], []

        def pop_one():
            if crit:
                crit.pop(0)[1]()
            elif defer:
                defer.pop(0)[1]()

        def flush(label):
            idxs = [i for i, (lb, _) in enumerate(crit) if lb == label]
            if not idxs:
                return
            for _ in range(idxs[-1] + 1):
                crit.pop(0)[1]()

        proj_k(0)
        proj_q(0, 0)
        for s in range(NB):
            qstate, vstate = {}, {}
            nstate = {}

            def vtasks(s=s, vstate=vstate):
                for t4 in range(4):
                    for h in range(2):
                        crit.append((f"v{t4}_{s}", lambda t4=t4, h=h:
                                     proj_v_half(s, t4, h, vstate)))

            def qtasks(c, s=s, qstate=qstate):
                for qd in range(4):
                    crit.append((f"q{c}_{s}", lambda c=c, qd=qd:
                                 proj_q_quad(s, c, qd, qstate)))

            if s == 0:
                vtasks()
                qtasks(1)
            else:
                qtasks(1)
                vtasks()
            qtasks(2)
            qtasks(3)
            if s + 1 < NB:
                for qd in range(4):
                    crit.append((f"k_{s + 1}", lambda qd=qd, s1=s + 1,
                                 nst=nstate: proj_k_quad(s1, qd, nst)))
                for qd in range(4):
                    crit.append((f"q0_{s + 1}", lambda qd=qd, s1=s + 1,
                                 nst=nstate: proj_q_quad(s1, 0, qd, nst)))
            if s >= 1:
                for u in range(16):
                    defer.append(("o", lambda u=u, s=s:
                                  outproj_unit(4 * (s - 1) + u // 4,
                                               u % 4)))

            def fill_one(j, s=s):
                if j >= 4 * s:
                    # diagonal k-tile j needs this window's V block j-4s
                    flush(f"v{j - 4 * s}_{s}")
                if s == NB - 1 and j % 2 == 1:
                    # last window is ACT-bound: don't oversubscribe PE
                    return
                pop_one()

            flush(f"q0_{s}")  # leftovers of this window's K/Q0 tasks
            attn_chunk(s, 0, fill=fill_one)
            flush(f"q1_{s}")
            attn_chunk(s, 1, fill=fill_one)
            flush(f"q2_{s}")
            attn_chunk(s, 2, fill=fill_one)
            flush(f"q3_{s}")
            attn_chunk(s, 3, last=(s == NB - 1), fill=fill_one)
            if s == 0:
                # wo needed from outproj(0) in the s=1 window
                nc.sync.dma_start(wo_sb[:, 0:2, :], wo[:, 0:2, :])
                nc.sync.dma_start(wo_sb[:, 2:4, :], wo[:, 2:4, :])
            if s + 2 < NB:
                load_xt(s + 2)
        while crit or defer:
            pop_one()
        for c in range(QC):
            for ob in range(4):
                outproj_unit(4 * (NB - 1) + c, ob, tail=True)

        if dumps is not None:
            for c in range(QC):
                dq = stage_pool.tile([P, S], F32, tag="dump", bufs=1,
                                     name=f"dq{c}")
                nc.vector.tensor_copy(dq[:], qt_sb[c][:])
                nc.sync.dma_start(dumps["qtd"][c * P:(c + 1) * P, :], dq[:])
                da = stage_pool.tile([P, S], F32, tag="dump", bufs=1,
                                     name=f"da{c}")
                nc.vector.tensor_copy(da[:], attn_sb[c][:])
                nc.sync.dma_start(dumps["ad"][c * P:(c + 1) * P, :], da[:])
            dk = stage_pool.tile([P, S], F32, tag="dump", bufs=1,
                                 name="dk")
            nc.vector.tensor_copy(dk[:], kt_sb[:])
            nc.sync.dma_start(dumps["ktd"][:], dk[:])
            for k in range(DC):
                dv = stage_pool.tile([P, 2 * (HD + 1)], F32, tag="dump",
                                     bufs=1, name=f"dv{k}")
                nc.vector.tensor_copy(dv[:], vaug_sb[k][:])
                nc.sync.dma_start(dumps["vd"][k * P:(k + 1) * P, :], dv[:])


_NC_CACHE = None


def _get_nc():
    global _NC_CACHE
    if _NC_CACHE is None:
        _NC_CACHE = build_kernel()
    return _NC_CACHE


def _deinterleave_cols(w):
    """Per 64-col head block: reorder cols to [evens(real), odds(imag)]."""
    d, n = w.shape
    out = np.empty_like(w)
    for h0 in range(0, n, HD):
        blk = w[:, h0:h0 + HD]
        out[:, h0:h0 + HD // 2] = blk[:, 0::2]
        out[:, h0 + HD // 2:h0 + HD] = blk[:, 1::2]
    return out


def _chunk_major(w, ncols):
    """(D_rows, ncols) -> (128, D_rows//128, ncols) partition-major bf16."""
    d = w.shape[0]
    return np.ascontiguousarray(
        w.reshape(d // P, P, ncols).transpose(1, 0, 2)).astype(NPBF16)


def _prep_inputs(x, wq, wk, wv, wo, freqs_cos, freqs_sin):
    scale = 1.0 / np.sqrt(HD)
    cosT = np.ascontiguousarray(freqs_cos[:S].T.astype(np.float32))  # (32,S)
    sinT = np.ascontiguousarray(freqs_sin[:S].T.astype(np.float32))
    c4 = np.tile(cosT, (4, 1)).astype(NPBF16)                  # (128, S)
    s4 = np.concatenate([-sinT, sinT, -sinT, sinT], 0).astype(NPBF16)
    kk = np.arange(P, dtype=np.int64)[:, None]
    qq = np.arange(SB, dtype=np.int64)[None, :]
    cmask = np.stack(
        [(kk <= qq - P * m).astype(np.float32) for m in range(4)]
    ).astype(NPBF16)

    xTs = [_chunk_major(np.ascontiguousarray(x[b].T), S) for b in range(B)]
    per_group = []
    for g in range(GROUPS):
        wq_full = np.ascontiguousarray(wq[:, g * QD:(g + 1) * QD])
        # chunk c holds heads [c, c+4] so q-head halves align with kv halves
        order = []
        for c in range(QC):
            order.extend(range(c * HD, (c + 1) * HD))
            order.extend(range((c + 4) * HD, (c + 5) * HD))
        wq_g = _deinterleave_cols(wq_full[:, order]) * scale
        wk_g = _deinterleave_cols(
            np.ascontiguousarray(wk[:, g * KD:(g + 1) * KD]))
        wv_g = np.ascontiguousarray(wv[:, g * KD:(g + 1) * KD])
        wkv_g = np.concatenate([wk_g, wv_g], axis=1)
        wo_g = np.ascontiguousarray(wo[g * QD:(g + 1) * QD, :][order, :])
        per_group.append((_chunk_major(wq_g, QD),
                          _chunk_major(wkv_g, KD + KD),
                          _chunk_major(wo_g, D)))

    in_maps = []
    for core in range(8):
        b, g = core // GROUPS, core % GROUPS
        wq_g, wkv_g, wo_g = per_group[g]
        in_maps.append({
            "xT": xTs[b],
            "wq": wq_g,
            "wkv": wkv_g,
            "wo": wo_g,
            "c4": c4,
            "s4": s4,
            "cmask": cmask,
        })
    return in_maps


def kernel(x, wq, wk, wv, wo, freqs_cos, freqs_sin, _trace=False):
    nc = _get_nc()
    in_maps = _prep_inputs(np.asarray(x, dtype=np.float32),
                           np.asarray(wq, dtype=np.float32),
                           np.asarray(wk, dtype=np.float32),
                           np.asarray(wv, dtype=np.float32),
                           np.asarray(wo, dtype=np.float32),
                           np.asarray(freqs_cos, dtype=np.float32),
                           np.asarray(freqs_sin, dtype=np.float32))
    res = run_bass_kernel_spmd(nc, in_maps, core_ids=list(range(8)),
                               trace=_trace)
    out = np.zeros((B, S, D), dtype=np.float32)
    for core in range(8):
        out[core // GROUPS] += res.results[core]["outp"].astype(np.float32)
    if _trace:
        kernel.last_results = res
    return out
